# revision 5
# baseline (speedup 1.0000x reference)
"""ObjectAttentionBlock Trainium2 Bass kernel — fp8 DoubleRow conv variant.

Same algorithm as kernel_bf16.py (conv6 folded into the value matrix),
with the two big pixel convolutions (f_pixel's conv0/conv1) run as
fp8-e4m3 DoubleRow matmuls: each matmul contracts 256 input channels
(2 fp8 weights per PE cell), halving the tensor-engine cycles of the
dominant GEMMs. Layout:
  - x is uploaded as fp8 tiles [128, 2, T]: partition p, subrow j
    holds channel (kt*256 + j*128 + p).
  - conv0/conv1 weights are uploaded pre-interleaved [128, 2, C]:
    [p, j, o*128+m] = w_eff.T[kt*256 + j*128 + p, o*128+m].
  - conv0's ReLU eviction writes fp8 into kt-paired 3D tiles that are
    directly the DoubleRow moving operand of conv1.
Every matmul runs fp8 DoubleRow: the similarity q^T key (q/key evicted
as fp8 in the kt-paired layout), the softmax row-sum (ones matrix), and
the output contraction against V2. The row-sum consumes the SAME fp8
quantized exp() tile as the numerator, so the normalization error
largely cancels. PSUM/eviction structure is the proven single-bank
rotation (4 conv + 4 attention banks); ReLU evictions split 6:2 over
ACT:DVE. numpy-sim end-to-end error ~6.2e-3 of scale (gate: 2e-2).
"""

import numpy as np
import ml_dtypes

import concourse.bass as bass
import concourse.mybir as mybir
import concourse.tile as tile
from concourse import bacc, bass_utils

N = 8
C = 512
K = 256
H = 128
W_IMG = 128
HW = H * W_IMG
P = 128          # partition width
CT = C // P      # 4 channel tiles
KT = K // P      # 2 region tiles
DT = 2           # DoubleRow kt groups (C / 256)
T = 512          # pixel tile (matmul moving dim / one PSUM bank of fp32)
NT = HW // T     # 32 pixel tiles
ALPHA = 1.0 / float(np.sqrt(C))

f32 = mybir.dt.float32
bf16 = mybir.dt.bfloat16
fp8 = mybir.dt.float8e4
AF = mybir.ActivationFunctionType
DR = mybir.MatmulPerfMode.DoubleRow
NP_BF16 = ml_dtypes.bfloat16
NP_FP8 = mybir.dt.np(fp8)


def build_module(n_tiles=NT):
    """Build and compile the per-core Bass module (SPMD: same on all cores)."""
    nc = bacc.Bacc("TRN2", target_bir_lowering=False, debug=False)
    xin = nc.dram_tensor("xin", [C, HW], fp8, kind="ExternalInput").ap()
    pin = nc.dram_tensor("pin", [C, K], bf16, kind="ExternalInput").ap()
    wdr_d = nc.dram_tensor("wdr", [2, DT, P, 2, C], fp8, kind="ExternalInput").ap()
    wt = nc.dram_tensor("wt", [5, C, C], bf16, kind="ExternalInput").ap()
    sbc_d = nc.dram_tensor("sbc", [P, 28], f32, kind="ExternalInput").ap()
    b6bc_d = nc.dram_tensor("b6bc", [P, C], f32, kind="ExternalInput").ap()
    onesdr_d = nc.dram_tensor("onesdr", [P, 2, P], fp8, kind="ExternalInput").ap()
    out_d = nc.dram_tensor("out", [C, HW], f32, kind="ExternalOutput").ap()

    with tile.TileContext(nc) as tc:
        with (
            tc.tile_pool(name="const", bufs=1) as cpool,
            tc.tile_pool(name="loop", bufs=2) as lpool,
            tc.tile_pool(name="psc", bufs=4, space="PSUM") as psp,
            tc.tile_pool(name="psa", bufs=4, space="PSUM") as psa,
        ):
            # ---- constants ----
            # Startup latency matters: spread the first-needed inputs over
            # four independent DMA queues so the PE can start ~2us in.
            sbc = cpool.tile([P, 28], f32, name="sbc_t")
            nc.gpsimd.dma_start(sbc[:], sbc_d[:])
            b6bc = cpool.tile([P, C], f32, name="b6bc_t")
            nc.scalar.dma_start(b6bc[:], b6bc_d[:])
            onesdr = cpool.tile([P, 2, P], fp8, name="onesdr_t")
            nc.scalar.dma_start(onesdr[:], onesdr_d[:])
            xt0 = [
                lpool.tile([P, 2, T], fp8, name=f"xt{kt}", tag=f"xt{kt}")
                for kt in range(DT)
            ]
            for kt in range(DT):
                for j in range(2):
                    nc.sync.dma_start(
                        xt0[kt][:, j, :],
                        xin[kt * 256 + j * P : kt * 256 + (j + 1) * P, 0:T],
                    )
            # DoubleRow weights for conv0/conv1: conv0 on the sync queue
            # right behind xt0, conv1 in parallel on the scalar queue.
            wdr = [
                [cpool.tile([P, 2, C], fp8, name=f"wdr{i}_{kt}") for kt in range(DT)]
                for i in range(2)
            ]
            for kt in range(DT):
                nc.sync.dma_start(wdr[0][kt][:], wdr_d[0, kt])
            for kt in range(DT):
                nc.scalar.dma_start(wdr[1][kt][:], wdr_d[1, kt])
            # bf16 weights for the proxy convs + V2 fold (w index: layer 2..6)
            w = [
                [cpool.tile([P, C], bf16, name=f"w{i}_{c}") for c in range(CT)]
                for i in range(5)
            ]
            p_t = [cpool.tile([P, K], bf16, name=f"p{c}") for c in range(CT)]
            for c in range(CT):
                nc.gpsimd.dma_start(p_t[c][:], pin[c * P : (c + 1) * P, :])
            for i in (0, 2, 1, 3, 4):  # dram layer order: 2,3,4,5,6
                for c in range(CT):
                    nc.gpsimd.dma_start(
                        w[i][c][:], wt[i, c * P : (c + 1) * P, :]
                    )

            def bias_ap(i, o):
                return sbc[:, i * 4 + o : i * 4 + o + 1]

            def _relu_evict(dst, ps, wi, o):
                # 3 of 4 conv evictions on ACT, 1 on DVE: keeps both
                # engines well under the PE streaming time.
                if o != 3:
                    nc.scalar.activation(dst, ps[:], AF.Relu, bias=bias_ap(wi, o))
                else:
                    nc.vector.tensor_scalar(
                        out=dst,
                        in0=ps[:],
                        scalar1=bias_ap(wi, o),
                        scalar2=0.0,
                        op0=mybir.AluOpType.add,
                        op1=mybir.AluOpType.max,
                    )

            def conv(inp, wi, outt, ncols, evict=None):
                """bf16 conv: outt[o] = relu(W@inp + b). wi is dram-layer-2-based."""
                for o in range(CT):
                    ps = psp.tile([P, ncols], f32, name=f"ps_{wi}_{o}", tag="cps")
                    for c in range(CT):
                        nc.tensor.matmul(
                            ps[:],
                            w[wi][c][:, o * P : (o + 1) * P],
                            inp[c][:],
                            start=(c == 0),
                            stop=(c == CT - 1),
                        )
                    if evict is not None:
                        evict(o, ps)
                    else:
                        nc.scalar.activation(
                            outt[o][:], ps[:], AF.Relu, bias=bias_ap(wi + 2, o)
                        )

            def conv_dr(inp, wi, evict):
                """fp8 DoubleRow conv over pixel tiles: inp = DT tiles [P,2,T].

                evict(o, ps) consumes the [P, T] psum of output block o."""
                for o in range(CT):
                    ps = psp.tile([P, T], f32, name=f"psd_{wi}_{o}", tag="cps")
                    for kt in range(DT):
                        nc.tensor.matmul(
                            ps[:],
                            wdr[wi][kt][:, :, o * P : (o + 1) * P],
                            inp[kt][:],
                            start=(kt == 0),
                            stop=(kt == DT - 1),
                            perf_mode=DR,
                        )
                    evict(o, ps)

            # ---- main pipeline over pixel tiles ----
            def stage_conv0(t, xt=None):
                if xt is None:
                    xt = [
                        lpool.tile([P, 2, T], fp8, name=f"xt{kt}", tag=f"xt{kt}")
                        for kt in range(DT)
                    ]
                    for kt in range(DT):
                        for j in range(2):
                            nc.sync.dma_start(
                                xt[kt][:, j, :],
                                xin[
                                    kt * 256 + j * P : kt * 256 + (j + 1) * P,
                                    t * T : (t + 1) * T,
                                ],
                            )
                t1 = [
                    lpool.tile([P, 2, T], fp8, name=f"t1_{kt}", tag=f"t1{kt}")
                    for kt in range(DT)
                ]
                conv_dr(xt, 0, lambda o, ps: _relu_evict(t1[o // 2][:, o % 2, :], ps, 0, o))
                return t1

            def stage_conv1(t1):
                q_dr = [
                    lpool.tile([P, 2, T], fp8, name=f"qdr{kt}", tag=f"qdr{kt}")
                    for kt in range(DT)
                ]
                conv_dr(t1, 1, lambda o, ps: _relu_evict(q_dr[o // 2][:, o % 2, :], ps, 1, o))
                return q_dr

            def stage_a_sim(q_dr):
                probT = lpool.tile([P, 2, T], fp8, name="pT", tag="pT")
                for k in range(KT):
                    ps = psp.tile([P, T], f32, name=f"ps_simT{k}", tag="cps")
                    for kt in range(DT):
                        nc.tensor.matmul(
                            ps[:],
                            key_dr[kt][:, :, k * P : (k + 1) * P],
                            q_dr[kt][:],
                            start=(kt == 0),
                            stop=(kt == DT - 1),
                            perf_mode=DR,
                        )
                    nc.scalar.activation(
                        probT[:, k, :], ps[:], AF.Exp, scale=ALPHA
                    )
                return probT

            def stage_b(t, probT):
                ps_rs = psa.tile([P, T], f32, name="ps_rs", tag="aps")
                nc.tensor.matmul(
                    ps_rs[:],
                    onesdr[:],
                    probT[:],
                    start=True,
                    stop=True,
                    perf_mode=DR,
                )
                rc = lpool.tile([P, T], f32, name="rc", tag="rc")
                nc.vector.reciprocal_approx_fast(out=rc[:], in_=ps_rs[:])
                outt = [
                    lpool.tile([P, T], f32, name=f"ot{o}", tag=f"ot{o}") for o in range(CT)
                ]
                for o in range(CT):
                    ps = psa.tile([P, T], f32, name=f"ps_out{o}", tag="aps")
                    nc.tensor.matmul(
                        ps[:],
                        v2T_dr[:, :, o * P : (o + 1) * P],
                        probT[:],
                        start=True,
                        stop=True,
                        perf_mode=DR,
                    )
                    # out = relu(acc * rc) == max(acc, 0) * rc  (rc > 0)
                    nc.vector.scalar_tensor_tensor(
                        out=outt[o][:],
                        in0=ps[:],
                        scalar=0.0,
                        in1=rc[:],
                        op0=mybir.AluOpType.max,
                        op1=mybir.AluOpType.mult,
                    )
                for o in range(CT):
                    eng = nc.sync if o < 2 else nc.gpsimd
                    eng.dma_start(
                        out_d[o * P : (o + 1) * P, t * T : (t + 1) * T], outt[o][:]
                    )

            q0 = stage_conv1(stage_conv0(0, xt=xt0))

            # ---- setup: key / folded-value from proxy ----
            # key is evicted as fp8 in the kt-paired DoubleRow layout so the
            # similarity matmul q^T key runs DoubleRow too.
            key_dr = [cpool.tile([P, 2, K], fp8, name=f"keydr{kt}") for kt in range(DT)]
            v2T_dr = cpool.tile([P, 2, C], fp8, name="v2Tdr")
            with tc.tile_pool(name="setup", bufs=1) as spool:
                k1 = [spool.tile([P, K], bf16, name=f"k1_{c}") for c in range(CT)]
                conv(p_t, 0, k1, K)
                conv(
                    k1,
                    1,
                    None,
                    K,
                    evict=lambda o, ps: nc.scalar.activation(
                        key_dr[o // 2][:, o % 2, :],
                        ps[:],
                        AF.Relu,
                        bias=bias_ap(3, o),
                    ),
                )
                v1 = [spool.tile([P, K], bf16, name=f"v1_{c}") for c in range(CT)]
                conv(p_t, 2, v1, K)
                val = [spool.tile([P, K], bf16, name=f"val{c}") for c in range(CT)]
                conv(v1, 3, val, K)
                # v2T[k] = val^T @ (s6 W6)^T + b6 row  (i.e. V2^T blocks)
                for k in range(KT):
                    pt = psa.tile([P, C], f32, name=f"ptv{k}", tag="aps")
                    for c in range(CT):
                        nc.tensor.matmul(
                            pt[:],
                            val[c][:, k * P : (k + 1) * P],
                            w[4][c][:],
                            start=(c == 0),
                            stop=(c == CT - 1),
                        )
                    nc.vector.tensor_tensor(
                        out=v2T_dr[:, k, :],
                        in0=pt[:],
                        in1=b6bc[:],
                        op=mybir.AluOpType.add,
                    )


            prev = None
            qcur = q0
            for t in range(n_tiles):
                t1n = stage_conv0(t + 1) if t + 1 < n_tiles else None
                pT = stage_a_sim(qcur)
                if t1n is not None:
                    qcur = stage_conv1(t1n)
                if prev is not None:
                    stage_b(prev[0], prev[1])
                prev = (t, pT)
            stage_b(prev[0], prev[1])

    nc.compile()
    return nc


def make_in_maps(x, proxy, W, s, b):
    # s > 0, so relu(s*(W@x)+b) == relu((diag(s)W)@x + b): fold s into W.
    w_eff = s[:, :, None].astype(np.float64) * W.astype(np.float64)
    wt_full = np.ascontiguousarray(w_eff.transpose(0, 2, 1))  # [7, c, o]
    # DoubleRow interleaved fp8 weights for conv0/conv1:
    # wdr[i, kt, p, j, o] = wt[i, kt*256 + j*128 + p, o]
    wdr = np.ascontiguousarray(
        wt_full[:2].reshape(2, DT, 2, P, C).transpose(0, 1, 3, 2, 4)
    ).astype(NP_FP8)
    wt = np.ascontiguousarray(wt_full[2:]).astype(NP_BF16)  # layers 2..6
    sbc = np.ascontiguousarray(
        b.reshape(7, CT, P).transpose(2, 0, 1).reshape(P, 7 * CT)
    ).astype(np.float32)
    b6bc = np.broadcast_to(b[6].astype(np.float32)[None, :], (P, C)).copy()
    onesdr = np.ones((P, 2, P), dtype=NP_FP8)
    in_maps = []
    for n in range(N):
        in_maps.append(
            {
                "xin": np.ascontiguousarray(x[n].reshape(C, HW)).astype(NP_FP8),
                "pin": np.ascontiguousarray(proxy[n].reshape(C, K)).astype(NP_BF16),
                "wdr": wdr,
                "wt": wt,
                "sbc": sbc,
                "b6bc": b6bc,
                "onesdr": onesdr,
            }
        )
    return in_maps


_CACHED = {}


def _get_module():
    if "nc" not in _CACHED:
        _CACHED["nc"] = build_module()
    return _CACHED["nc"]


def kernel(x, proxy, W, s, b):
    nc = _get_module()
    in_maps = make_in_maps(x, proxy, W, s, b)
    res = bass_utils.run_bass_kernel_spmd(nc, in_maps, core_ids=list(range(N)))
    out = np.stack([res.results[n]["out"].reshape(C, H, W_IMG) for n in range(N)])
    return out.astype(np.float32)
